# revision 4
# baseline (speedup 1.0000x reference)
"""BiRNN (Bowman SNLI) Trainium2 kernel — transposed-state formulation.

Full inputs -> full logits [256, 3].

Sharding: 8 cores = 2 batch halves x 4 LSTM runs (p_fw, p_bw, h_fw, h_bw).
Each core runs one masked-LSTM direction over its 128-row batch half.

Key ideas vs the v1 kernel:
- State is kept transposed: c^T, h^T are [H-chunk partitions, batch cols].
  The gate matmuls z^T = Wx^T x^T + Wh^T h^T put BATCH in the moving (N)
  dim, so per-step cost scales with the number of still-active rows.
- Rows are sorted by max(premise_len, hypothesis_len) descending and dealt
  round-robin into the two halves; the per-step active count ks[t] is baked
  into the program (built per input data), shrinking matmul/activation work
  as sequences finish. Expected ~1.5-2x column reduction for uniform lengths.
- All matmul operands are bf16 (1 cycle/row at any N; fp32 PSUM accumulate).
- No transposes: h^T is produced directly by the elementwise chain, which
  runs on strided [128, 4, k] views so frozen columns are never touched.
- Sequence masking is an extra K-row of x^T (1.0 when frozen) whose weight
  row is -BIG on the i gate and +BIG on the f gate => c frozen exactly.
- Backward direction: host packs x^T time-shifted per row (x[len-1-t]) so
  every row is active from step 0, same shrinking schedule as forward.
"""
import os
import sys
from contextlib import ExitStack

sys.path.insert(0, "/opt/trn_rl_repo")

import numpy as np

import concourse.bass as bass
import concourse.mybir as mybir
import concourse.tile as tile
from concourse import bacc
from concourse import bass_utils

f32 = mybir.dt.float32
bf16 = mybir.dt.bfloat16
AF = mybir.ActivationFunctionType

B = 256
T = int(os.environ.get("KBENCH_T", "256"))
D = 300
H = 512
FFD = 1024
FORGET_BIAS = 1.0
BIG = 30.0
NB = 128          # batch rows per core
G = 4 * H         # 2048 gate width
NKX = 3           # ceil(302/128) x-proj K chunks (300 x + bias + mask)
NKH = 4           # H/128 recurrent K chunks
KALIGN = 2

# gate column ranges in z/Wx/Wh: i, j, f, o (BasicLSTMCell order)
GI, GJ, GF, GO = 0, 1, 2, 3


def build(ks=None, with_ff=True, repeat=1):
    """ks: list of active-column counts per step (non-increasing, <=128).
    Defaults to the schedule computed by the last make_in_maps() call."""
    ks = list(ks if ks is not None else _SCHED)
    Ts = len(ks)
    offs = np.concatenate([[0], np.cumsum(ks)]).astype(int)
    SUMK = int(offs[-1])

    nc = bacc.Bacc("TRN2", num_devices=8)

    xTd = nc.dram_tensor("xT", [128, NKX, SUMK], bf16, kind="ExternalInput")
    pmd = nc.dram_tensor("pm", [128, 128], bf16, kind="ExternalInput")
    identd = nc.dram_tensor("identd", [128, 128], bf16, kind="ExternalInput")
    wxd = nc.dram_tensor("wx", [128, NKX * G], bf16, kind="ExternalInput")
    whd = nc.dram_tensor("wh", [128, NKH * G], bf16, kind="ExternalInput")
    w1d = nc.dram_tensor("w1", [128, 16 * FFD], bf16, kind="ExternalInput")
    w2d = nc.dram_tensor("w2", [128, 8 * FFD], bf16, kind="ExternalInput")
    w3d = nc.dram_tensor("w3", [128, 8 * FFD], bf16, kind="ExternalInput")
    w4d = nc.dram_tensor("w4", [128, 8 * 4], bf16, kind="ExternalInput")
    bffd = nc.dram_tensor("bff", [128, 24], f32, kind="ExternalInput")
    b4d = nc.dram_tensor("b4r", [1, 4], bf16, kind="ExternalInput")
    onesd = nc.dram_tensor("onesd", [1, 128], bf16, kind="ExternalInput")

    coutd = nc.dram_tensor("cout", [128, H], f32, kind="ExternalOutput")
    logitsd = nc.dram_tensor("logits", [4, 128], f32, kind="ExternalOutput")
    zdbgd = None
    if os.environ.get("KDEBUG_Z"):
        zdbgd = nc.dram_tensor("zdbg", [4, 128, 512], f32, kind="ExternalOutput")

    def v3(t_, k):
        """[128, 512]-tile view as [128, 4, k] (chunk-strided, k active cols)."""
        return t_[:].rearrange("p (q b) -> p q b", b=128)[:, :, 0:k]

    with tile.TileContext(nc) as tc, ExitStack() as es:
        kpool = es.enter_context(tc.tile_pool(name="keep", bufs=1))
        dpool = es.enter_context(tc.tile_pool(name="ffdram", bufs=1, space="DRAM"))

        ones1 = kpool.tile([1, 128], bf16)
        nc.sync.dma_start(ones1[:], onesd[:])

        ccd = dpool.tile([4, 128, 128], bf16)       # own c^T chunks
        cca = dpool.tile([4, 4, 128, 128], bf16)    # gathered group c^T

        w1pool = es.enter_context(tc.tile_pool(name="ffw1", bufs=1))
        w1t = w1pool.tile([128, 16 * FFD], bf16)
        if with_ff:
            for kk in range(4):
                nc.sync.dma_start(w1t[:, kk * 4 * FFD:(kk + 1) * 4 * FFD],
                                  w1d[:, kk * 4 * FFD:(kk + 1) * 4 * FFD])

        lstm_es = ExitStack()
        cpool = lstm_es.enter_context(tc.tile_pool(name="const", bufs=1))
        spool = lstm_es.enter_context(tc.tile_pool(name="state", bufs=1))
        xpool = lstm_es.enter_context(tc.tile_pool(name="xin", bufs=4))
        zpool = lstm_es.enter_context(tc.tile_pool(name="zpsum", bufs=8, space="PSUM"))
        gpool = lstm_es.enter_context(tc.tile_pool(name="gact", bufs=8))
        tpool = lstm_es.enter_context(tc.tile_pool(name="tmp", bufs=6))

        wxt = cpool.tile([128, NKX * G], bf16)
        wht = cpool.tile([128, NKH * G], bf16)
        pmt = cpool.tile([128, 128], bf16)
        identt = cpool.tile([128, 128], bf16)
        nc.sync.dma_start(wxt[:], wxd[:])
        nc.sync.dma_start(wht[:], whd[:])
        nc.sync.dma_start(pmt[:], pmd[:])
        nc.sync.dma_start(identt[:], identd[:])

        cT = spool.tile([128, H], f32)
        hT = spool.tile([128, H], bf16)

        xq = {}

        def emit_xdma(t):
            """Issue the x^T(t) load into a fresh ring buffer (prefetch)."""
            k = ks[t]
            off = int(offs[t])
            xt = xpool.tile([128, NKX * 128], bf16, tag="xt", name=f"xt{t % 4}")
            nc.sync.dma_start(
                xt[:].rearrange("p (c b) -> p c b", b=128)[:, :, 0:k],
                xTd[:, :, off:off + k])
            xq[t] = xt

        def emit_xproj(t, final=False, skip_o=False):
            """Accumulate the x-projection for step t into fresh z banks."""
            k = ks[t]
            if t not in xq:
                emit_xdma(t)
            xt = xq.pop(t)
            z = [zpool.tile([128, 512], f32, tag="z", name=f"z{t % 2}_{g}")
                 for g in range(4)]
            ng = 3 if skip_o else 4
            for c in range(NKX):
                for g in range(ng):
                    for qq in range(4):
                        m = g * 4 + qq
                        # start clears has_written for the WHOLE bank: set it
                        # only on the first matmul into bank g this episode.
                        nc.tensor.matmul(
                            z[g][:, qq * 128:qq * 128 + k],
                            wxt[:, c * G + m * 128:c * G + (m + 1) * 128],
                            xt[:, c * 128:c * 128 + k],
                            start=(c == 0 and qq == 0),
                            stop=(final and c == NKX - 1 and qq == 3),
                        )
            return z

        def emit_hproj(t, z, skip_o=False):
            """Three phases: q01 for all regions (needs hT chunks 0-1 only),
            then q23 for regions qq01 (completes the A-half gates), then q23
            for regions qq23 (completes the B-half)."""
            k = ks[t]
            gorder = (GF, GI, GJ) if skip_o else (GF, GI, GJ, GO)

            def mm(g, qq, q, stop=False):
                m = g * 4 + qq
                nc.tensor.matmul(
                    z[g][:, qq * 128:qq * 128 + k],
                    wht[:, q * G + m * 128:q * G + (m + 1) * 128],
                    hT[:, q * 128:q * 128 + k],
                    start=False, stop=stop,
                )

            for g in gorder:
                for qq in range(4):
                    for q in range(NKH):
                        mm(g, qq, q, stop=(qq == 3 and q == NKH - 1))

        def vh(t_, k, hv):
            """[128, 512]-tile view of half hv (chunks 2hv..2hv+1), k cols."""
            return t_[:].rearrange("p (q b) -> p q b", b=128)[:, 2 * hv:2 * hv + 2, 0:k]

        def chain(t, z, last=False):
            k = ks[t]
            if t == 0 and zdbgd is not None:
                for g in range(4 if not last else 3):
                    zs = tpool.tile([128, H], f32, tag="zdbg", name=f"zs{g}")
                    nc.vector.tensor_copy(zs[:], z[g][:])
                    nc.sync.dma_start(zdbgd[g], zs[:])
            gf = gpool.tile([128, H], bf16, tag="g")
            gi = gpool.tile([128, H], bf16, tag="g")
            gj = gpool.tile([128, H], bf16, tag="g")
            nc.scalar.activation(v3(gf, k), v3(z[GF], k), AF.Sigmoid)
            nc.scalar.activation(v3(gi, k), v3(z[GI], k), AF.Sigmoid)
            nc.scalar.activation(v3(gj, k), v3(z[GJ], k), AF.Tanh)
            if t > 0:
                p2 = tpool.tile([128, H], f32, tag="t")
                nc.vector.tensor_mul(v3(p2, k), v3(cT, k), v3(gf, k))
            p1 = tpool.tile([128, H], bf16, tag="t")
            nc.vector.tensor_mul(v3(p1, k), v3(gi, k), v3(gj, k))
            if t > 0:
                nc.vector.tensor_add(v3(cT, k), v3(p1, k), v3(p2, k))
            else:
                nc.vector.tensor_copy(v3(cT, k), v3(p1, k))
            if not last:
                go = gpool.tile([128, H], bf16, tag="g")
                nc.scalar.activation(v3(go, k), v3(z[GO], k), AF.Sigmoid)
                th = tpool.tile([128, H], bf16, tag="t")
                nc.scalar.activation(v3(th, k), v3(cT, k), AF.Tanh)
                nc.vector.tensor_mul(v3(hT, k), v3(th, k), v3(go, k))

        def run_lstm():
            z = emit_xproj(0, final=True, skip_o=(Ts == 1))
            for t in range(Ts):
                last = t == Ts - 1
                if t > 0:
                    emit_hproj(t, z, skip_o=last)
                znext = None
                if not last:
                    znext = emit_xproj(t + 1, skip_o=(t + 1 == Ts - 1))
                chain(t, z, last=last)
                z = znext

            ctb = tpool.tile([128, H], bf16, tag="ctb")
            nc.vector.tensor_copy(ctb[:], cT[:])
            if os.environ.get("KDEBUG_C"):
                nc.sync.dma_start(coutd[:], cT[:])
            if with_ff:
                # Realign batch columns to the half's canonical order:
                # c_chunk = (cT_chunk)^T via identity, then cT_perm = c^T @ Pm.
                ctb2 = tpool.tile([128, H], bf16, tag="ctb2")
                for q in range(4):
                    ptA = zpool.tile([128, 512], f32, tag="z", name=f"ptA{q}")
                    nc.tensor.matmul(ptA[:, 0:128],
                                     ctb[:, q * 128:(q + 1) * 128],
                                     identt[:], start=True, stop=True)
                    cbs = tpool.tile([128, 128], bf16, tag="cbs")
                    nc.vector.tensor_copy(cbs[:], ptA[:, 0:128])
                    ptB = zpool.tile([128, 512], f32, tag="z", name=f"ptB{q}")
                    nc.tensor.matmul(ptB[:, 0:128], cbs[:], pmt[:],
                                     start=True, stop=True)
                    nc.vector.tensor_copy(
                        ctb2[:, q * 128:(q + 1) * 128], ptB[:, 0:128])
                nc.sync.dma_start(
                    ccd[:].rearrange("q p b -> p q b"),
                    ctb2[:].rearrange("p (q b) -> p q b", b=128))

        if repeat > 1:
            with tc.For_i(0, repeat, 1):
                run_lstm()
        else:
            run_lstm()

        lstm_es.close()
        if not with_ff:
            nc.compile()
            return nc

        # ---------------- FF head (transposed, bf16) ----------------
        nc.gpsimd.collective_compute(
            "AllGather", mybir.AluOpType.bypass,
            replica_groups=[[0, 1, 2, 3], [4, 5, 6, 7]],
            ins=[ccd.opt()], outs=[cca.opt()],
        )
        with tc.tile_pool(name="ffw", bufs=1) as fpool, \
             tc.tile_pool(name="ffa", bufs=2) as fapool, \
             tc.tile_pool(name="ffp", bufs=3, space="PSUM") as fppool, \
             tc.tile_pool(name="ffp4", bufs=1, space="PSUM") as fp4pool:
            w2t = fpool.tile([128, 8 * FFD], bf16)
            w3t = fpool.tile([128, 8 * FFD], bf16)
            w4t = fpool.tile([128, 8 * 4], bf16)
            bfft = fpool.tile([128, 24], f32)
            b4t = fpool.tile([1, 4], bf16)
            nc.sync.dma_start(w2t[:], w2d[:])
            nc.sync.dma_start(w3t[:], w3d[:])
            nc.sync.dma_start(w4t[:], w4d[:])
            nc.sync.dma_start(bfft[:], bffd[:])
            nc.sync.dma_start(b4t[:], b4d[:])

            def ff_layer(actT, nk, wt, boff, tag):
                """h' = tanh(W^T actT + b): actT [128, nk*128] -> [128, 8*128]."""
                outT = fapool.tile([128, 8 * 128], bf16, tag=tag)
                for half in range(2):
                    pg = fppool.tile([128, 512], f32, tag="ffp")
                    for mm in range(4):
                        m = half * 4 + mm
                        for q in range(nk):
                            nc.tensor.matmul(
                                pg[:, mm * 128:(mm + 1) * 128],
                                wt[:, q * FFD + m * 128:q * FFD + (m + 1) * 128],
                                actT[:, q * 128:(q + 1) * 128],
                                start=(q == 0), stop=(q == nk - 1),
                            )
                    for mm in range(4):
                        m = half * 4 + mm
                        nc.scalar.activation(
                            outT[:, m * 128:(m + 1) * 128],
                            pg[:, mm * 128:(mm + 1) * 128],
                            AF.Tanh, bias=bfft[:, boff + m:boff + m + 1])
                return outT

            def run_ff():
                xcatT = fapool.tile([128, 16 * 128], bf16, tag="xcatT")
                nc.sync.dma_start(
                    xcatT[:].rearrange("p (c b) -> p c b", b=128),
                    cca[:].rearrange("r q p b -> p (r q) b"))
                h1T = ff_layer(xcatT, 16, w1t, 0, "h1T")
                h2T = ff_layer(h1T, 8, w2t, 8, "h2T")
                h3T = ff_layer(h2T, 8, w3t, 16, "h3T")
                pg4 = fp4pool.tile([4, 128], f32, tag="ffp4")
                for q in range(8):
                    nc.tensor.matmul(
                        pg4[:], w4t[:, q * 4:(q + 1) * 4],
                        h3T[:, q * 128:(q + 1) * 128],
                        start=(q == 0), stop=False)
                nc.tensor.matmul(pg4[:], b4t[:], ones1[:],
                                 start=False, stop=True)
                lgt = fapool.tile([4, 128], f32, tag="lgt")
                nc.vector.tensor_copy(lgt[:], pg4[:])
                nc.sync.dma_start(logitsd[:], lgt[:])

            if repeat > 1:
                with tc.For_i(0, repeat, 1):
                    run_ff()
            else:
                run_ff()

    nc.compile()
    return nc


def _schedule(premise_len, hypothesis_len):
    """Shared half split; per-profile row order inside each half.

    Returns (rows_by_core4, canon, ks): rows_by_core4[half][run] is the
    column->global-row order for that core (run 0/1 premise, 2/3 hypothesis);
    canon[half] is the canonical (premise) order the FF columns follow.
    """
    plen = np.minimum(np.asarray(premise_len).astype(np.int64), T)
    hlen = np.minimum(np.asarray(hypothesis_len).astype(np.int64), T)
    maxlen = np.maximum(plen, hlen)
    order = np.argsort(-maxlen, kind="stable")
    halves = [order[0::2], order[1::2]]
    rows_by_core4 = []
    canon = []
    counts = []
    for h in halves:
        op = h[np.argsort(-plen[h], kind="stable")]
        oh = h[np.argsort(-hlen[h], kind="stable")]
        rows_by_core4.append([op, op, oh, oh])
        canon.append(op)
        counts.append((plen[h], hlen[h]))
    Ts = int(maxlen.max())
    ks = []
    for r in range(Ts):
        k = max(max(int((pl > r).sum()), int((hl > r).sum()))
                for pl, hl in counts)
        k = min(128, ((max(k, 1) + KALIGN - 1) // KALIGN) * KALIGN)
        ks.append(k)
    return rows_by_core4, canon, ks


def _pack_x(x_rows, eff_len, ks, reverse):
    """x_rows [128, T, D] f32, eff_len [128] -> xT [128, NKX, SUMK] bf16."""
    Ts = len(ks)
    offs = np.concatenate([[0], np.cumsum(ks)]).astype(int)
    out = np.zeros((128, NKX, int(offs[-1])), np.float32)
    Xp = np.zeros((NKX * 128, 128), np.float32)
    for r in range(Ts):
        k = int(ks[r])
        Xp[:, :] = 0.0
        active = r < eff_len            # [128]
        idx = np.where(active)[0]
        idx = idx[idx < k]
        if reverse:
            tsrc = eff_len[idx] - 1 - r
        else:
            tsrc = np.full(len(idx), r)
        Xp[:D, idx] = x_rows[idx, tsrc, :].T
        Xp[D, :k] = 1.0                                   # bias row
        Xp[D + 1, :k] = (~active[:k]).astype(np.float32)  # frozen mask row
        out[:, :, offs[r]:offs[r] + k] = \
            Xp.reshape(NKX, 128, 128)[:, :, :k].transpose(1, 0, 2)
    return out


def _to_bf16(a):
    import ml_dtypes
    return np.asarray(a, np.float32).astype(ml_dtypes.bfloat16)


def _pack_wx(Wx, b):
    Wxp = np.zeros((NKX * 128, G), np.float32)
    Wxp[:D] = Wx
    Wxp[D] = b
    Wxp[D, 2 * H:3 * H] += FORGET_BIAS
    Wxp[D + 1, 0:H] = -BIG            # i gate: frozen -> sigmoid ~ 0
    Wxp[D + 1, 2 * H:3 * H] = BIG     # f gate: frozen -> sigmoid ~ 1
    return Wxp.reshape(NKX, 128, G).transpose(1, 0, 2).reshape(128, NKX * G)


def _pack_wh(Wh):
    return Wh.reshape(NKH, 128, G).transpose(1, 0, 2).reshape(128, NKH * G)


def pack_core_inputs(x_half, len_half, Wx, Wh, b, reverse, ks,
                     W1, b1, W2, b2, W3, b3, W4, b4):
    eff_len = np.minimum(np.asarray(len_half).astype(np.int64), T)
    xT_ = _pack_x(np.asarray(x_half), eff_len, ks, reverse)

    w1_ = np.asarray(W1).reshape(16, 128, FFD).transpose(1, 0, 2).reshape(128, -1)
    w2_ = np.asarray(W2).reshape(8, 128, FFD).transpose(1, 0, 2).reshape(128, -1)
    w3_ = np.asarray(W3).reshape(8, 128, FFD).transpose(1, 0, 2).reshape(128, -1)
    w4p = np.zeros((FFD, 4), np.float32)
    w4p[:, :3] = np.asarray(W4)
    w4_ = w4p.reshape(8, 128, 4).transpose(1, 0, 2).reshape(128, -1)
    bff_ = np.zeros((128, 24), np.float32)
    bff_[:, 0:8] = np.asarray(b1).reshape(8, 128).T
    bff_[:, 8:16] = np.asarray(b2).reshape(8, 128).T
    bff_[:, 16:24] = np.asarray(b3).reshape(8, 128).T
    b4_ = np.zeros((1, 4), np.float32)
    b4_[0, :3] = np.asarray(b4)

    return {
        "xT": _to_bf16(xT_),
        "wx": _to_bf16(_pack_wx(np.asarray(Wx), np.asarray(b))),
        "wh": _to_bf16(_pack_wh(np.asarray(Wh))),
        "w1": _to_bf16(w1_), "w2": _to_bf16(w2_), "w3": _to_bf16(w3_),
        "w4": _to_bf16(w4_), "bff": bff_.astype(np.float32),
        "b4r": _to_bf16(b4_), "onesd": _to_bf16(np.ones((1, 128), np.float32)),
    }


def make_in_maps(premises, hypotheses, premise_len, hypothesis_len,
                 p_fw_Wx, p_fw_Wh, p_fw_b, p_bw_Wx, p_bw_Wh, p_bw_b,
                 h_fw_Wx, h_fw_Wh, h_fw_b, h_bw_Wx, h_bw_Wh, h_bw_b,
                 W1, b1, W2, b2, W3, b3, W4, b4):
    premises = np.asarray(premises)[:, :T, :]
    hypotheses = np.asarray(hypotheses)[:, :T, :]
    rows_by_core4, canon, ks = _schedule(premise_len, hypothesis_len)
    global _SCHED, _CANON
    _SCHED = ks
    _CANON = canon
    ff = (W1, b1, W2, b2, W3, b3, W4, b4)
    in_maps = []
    for half in range(2):
        for run, (x, ln, Wx_, Wh_, b_, rev) in enumerate([
            (premises, premise_len, p_fw_Wx, p_fw_Wh, p_fw_b, False),
            (premises, premise_len, p_bw_Wx, p_bw_Wh, p_bw_b, True),
            (hypotheses, hypothesis_len, h_fw_Wx, h_fw_Wh, h_fw_b, False),
            (hypotheses, hypothesis_len, h_bw_Wx, h_bw_Wh, h_bw_b, True),
        ]):
            rows = rows_by_core4[half][run]
            im = pack_core_inputs(
                np.asarray(x)[rows], np.asarray(ln)[rows],
                np.asarray(Wx_), np.asarray(Wh_), np.asarray(b_), rev, ks, *ff)
            # Pm[b, b'] = 1 iff this core's column b is canonical column b'.
            pm = np.zeros((128, 128), np.float32)
            pos = {r: i for i, r in enumerate(canon[half])}
            for b, r in enumerate(rows):
                pm[b, pos[r]] = 1.0
            im["pm"] = _to_bf16(pm)
            im["identd"] = _to_bf16(np.eye(128, dtype=np.float32))
            in_maps.append(im)
    return in_maps


_SCHED = None
_CANON = None
_NC_CACHE = {}


def get_nc(with_ff=True):
    key = (T, with_ff, tuple(_SCHED))
    if key not in _NC_CACHE:
        _NC_CACHE[key] = build(_SCHED, with_ff=with_ff)
    return _NC_CACHE[key]


def kernel(**inputs):
    in_maps = make_in_maps(**inputs)
    nc = get_nc()
    res = bass_utils.run_bass_kernel_spmd(nc, in_maps, core_ids=list(range(8)))
    out = np.empty((B, 3), np.float32)
    out[_CANON[0]] = res.results[0]["logits"][:3, :].T
    out[_CANON[1]] = res.results[4]["logits"][:3, :].T
    kernel.last_results = res
    return out
